# revision 1
# baseline (speedup 1.0000x reference)
"""Causal multi-head attention on 8 Trainium2 NeuronCores.

Sharding: core c -> (batch g = c // 4, head-group p = c % 4, heads 4p..4p+3).
Each core projects Q/K/V for its batch with its 256 feature columns
(column-sharded w_q/w_k/w_v), runs causal attention for its 4 heads in
transposed (scores.T) layout with an augmented-ones column on V to get the
softmax denominators for free, computes the partial output projection with
its 256 rows of w_o, and a ReduceScatter over each batch group sums the
partials and hands every core its own 512-row output shard.

Matmuls run as float32r (full PE rate, ~1.5e-4 rel err); accumulation fp32.
"""

import numpy as np

B, S, D, H = 2, 2048, 1024, 16
DK = D // H  # 64
N_CORES = 8
FPC = 256  # features per core

_CACHE = {}


def _build_nc():
    import os as os_mod
    import concourse.mybir as mybir
    import concourse.tile as tile
    from concourse import bacc

    F32 = mybir.dt.float32
    F32R = mybir.dt.float32r
    BF16 = mybir.dt.bfloat16
    x_bf16 = bool(os_mod.environ.get("BASS_X_BF16"))
    XD = BF16 if x_bf16 else F32
    XDR = BF16 if x_bf16 else F32R
    Exp = mybir.ActivationFunctionType.Exp

    nc = bacc.Bacc("TRN2", target_bir_lowering=False, debug=False, num_devices=8)

    xq = nc.dram_tensor("xq", [D, S], XD, kind="ExternalInput")
    xk = nc.dram_tensor("xk", [D, S], XD, kind="ExternalInput")
    xv = nc.dram_tensor("xv", [D, S], XD, kind="ExternalInput")
    wq = nc.dram_tensor("wq", [D, FPC], XD, kind="ExternalInput")
    wk = nc.dram_tensor("wk", [D, FPC], XD, kind="ExternalInput")
    wv = nc.dram_tensor("wv", [D, FPC], XD, kind="ExternalInput")
    wo = nc.dram_tensor("wo", [FPC, D], F32, kind="ExternalInput")
    bq = nc.dram_tensor("bq", [FPC, 1], F32, kind="ExternalInput")
    bk = nc.dram_tensor("bk", [FPC, 1], F32, kind="ExternalInput")
    bv = nc.dram_tensor("bv", [FPC, 1], F32, kind="ExternalInput")
    bo4 = nc.dram_tensor("bo4", [128, D], F32, kind="ExternalInput")
    masks = nc.dram_tensor("masks", [128, 2048], F32, kind="ExternalInput")
    ident = nc.dram_tensor("ident", [128, 128], F32, kind="ExternalInput")
    out = nc.dram_tensor("out", [512, D], F32, kind="ExternalOutput")

    NKT = S // 128  # 16 kpos tiles
    NQB = S // 512  # 4 q blocks

    from contextlib import ExitStack
    stack = ExitStack()
    with tile.TileContext(nc) as tc:
        with (
            tc.tile_pool(name="consts", bufs=1) as consts,
            tc.tile_pool(name="persist", bufs=1) as persist,
            tc.tile_pool(name="xin", bufs=3) as xin,
            tc.tile_pool(name="probs", bufs=4) as probs,
            tc.tile_pool(name="small", bufs=2) as small,
            tc.tile_pool(name="oout", bufs=3) as oout,
            tc.tile_pool(name="dram", bufs=1, space="DRAM") as dram,
        ):
            # ---- constants; only wq chunk0 + first x chunk gate startup ----
            wq_s = consts.tile([128, 8, FPC], XDR, tag="wq")
            wk_s = consts.tile([128, 8, FPC], XDR, tag="wk")
            wv_s = consts.tile([128, 8, FPC], XDR, tag="wv")
            wo_s = consts.tile([128, 2, D], F32R, tag="wo")
            masks_s = consts.tile([128, 2048], F32R, tag="masks")
            ident_s = consts.tile([128, 128], F32, tag="ident")
            bq_s = consts.tile([128, 2], F32, tag="bq")
            bk_s = consts.tile([128, 2], F32, tag="bk")
            bv_s = consts.tile([128, 2], F32, tag="bv")
            bo4_s = consts.tile([128, D], F32, tag="bo4")

            nc.sync.dma_start(wq_s[:, 0, :], wq[0:128, :].bitcast(XDR))
            xt0 = xin.tile([128, S], XDR, tag="x", name="xt0")
            for qb0 in range(4):
                nc.sync.dma_start(
                    xt0[:, 512 * qb0 : 512 * (qb0 + 1)],
                    xq[0:128, 512 * qb0 : 512 * (qb0 + 1)].bitcast(XDR))
            for kc in range(1, 8):
                nc.sync.dma_start(
                    wq_s[:, kc, :], wq[128 * kc : 128 * (kc + 1), :].bitcast(XDR))
            nc.sync.dma_start(bq_s[:], bq.ap().rearrange("(t p) o -> p (t o)", p=128))

            # ---- persistent activations ----
            qT_s = [persist.tile([128, S], F32R, tag=f"qT{i}", name=f"qT{i}") for i in range(2)]
            kT_s = [persist.tile([128, S], F32R, tag=f"kT{i}", name=f"kT{i}") for i in range(2)]
            v_s = persist.tile([128, NKT, 4 * 65], F32R, tag="v")
            ctx_s = [persist.tile([128, S], F32R, tag=f"ctx{i}", name=f"ctx{i}") for i in range(2)]

            # ones columns of V_aug (col 64 of each head's 65-wide strip)
            for h in range(4):
                nc.vector.memset(v_s[:, :, 65 * h + 64 : 65 * h + 65].bitcast(F32), 1.0)

            # ---- phase 1: projections (shared PSUM pool, slots recycle) ----
            with (
                tc.tile_pool(name="psProj", bufs=8, space="PSUM") as psP,
                tc.tile_pool(name="vtp", bufs=1) as vtp,
            ):
                # Q pass: psum [2pt x 4qb] accumulate over 8 k-chunks
                psq = {(pt, qb): psP.tile([128, 512], F32, tag="pp", name=f"psq{pt}{qb}")
                       for pt in range(2) for qb in range(NQB)}
                dma_eng = [nc.sync, nc.scalar, nc.gpsimd]
                for kc in range(8):
                    if kc == 0:
                        xt = xt0
                    else:
                        xt = xin.tile([128, S], XDR, tag="x")
                        dma_eng[kc % 3].dma_start(xt[:], xq[128 * kc : 128 * (kc + 1), :].bitcast(XDR))
                    for pt in range(2):
                        for qb in range(NQB):
                            nc.tensor.matmul(
                                psq[(pt, qb)][:],
                                wq_s[:, kc, 128 * pt : 128 * (pt + 1)],
                                xt[:, 512 * qb : 512 * (qb + 1)],
                                start=(kc == 0), stop=(kc == 7),
                            )
                for pt in range(2):
                    for qb in range(NQB):
                        nc.vector.tensor_scalar_add(
                            qT_s[pt][:, 512 * qb : 512 * (qb + 1)],
                            psq[(pt, qb)][:], bq_s[:, pt : pt + 1],
                        )
                # K pass
                nc.sync.dma_start(wk_s[:], wk.ap().rearrange("(kc p) f -> p kc f", p=128).bitcast(XDR))
                nc.sync.dma_start(bk_s[:], bk.ap().rearrange("(t p) o -> p (t o)", p=128))
                psk = {(pt, qb): psP.tile([128, 512], F32, tag="pp", name=f"psk{pt}{qb}")
                       for pt in range(2) for qb in range(NQB)}
                for kc in range(8):
                    xt = xin.tile([128, S], XDR, tag="x")
                    dma_eng[kc % 3].dma_start(xt[:], xk[128 * kc : 128 * (kc + 1), :].bitcast(XDR))
                    for pt in range(2):
                        for qb in range(NQB):
                            nc.tensor.matmul(
                                psk[(pt, qb)][:],
                                wk_s[:, kc, 128 * pt : 128 * (pt + 1)],
                                xt[:, 512 * qb : 512 * (qb + 1)],
                                start=(kc == 0), stop=(kc == 7),
                            )
                for pt in range(2):
                    for qb in range(NQB):
                        nc.vector.tensor_scalar_add(
                            kT_s[pt][:, 512 * qb : 512 * (qb + 1)],
                            psk[(pt, qb)][:], bk_s[:, pt : pt + 1],
                        )
                # V pass: compute V.T like Q/K, then PE-transpose to natural
                nc.sync.dma_start(wv_s[:], wv.ap().rearrange("(kc p) f -> p kc f", p=128).bitcast(XDR))
                nc.sync.dma_start(bv_s[:], bv.ap().rearrange("(t p) o -> p (t o)", p=128))
                nc.sync.dma_start(masks_s[:], masks.ap().bitcast(F32R))
                nc.sync.dma_start(ident_s[:], ident.ap())
                vT_s = [vtp.tile([128, S], F32, tag=f"vT{i}", name=f"vT{i}") for i in range(2)]
                psv = {(pt, kb): psP.tile([128, 512], F32, tag="pp", name=f"psv{pt}{kb}")
                       for pt in range(2) for kb in range(NQB)}
                for kc in range(8):
                    xt = xin.tile([128, S], XDR, tag="x")
                    dma_eng[kc % 3].dma_start(xt[:], xv[128 * kc : 128 * (kc + 1), :].bitcast(XDR))
                    for pt in range(2):
                        for kb in range(NQB):
                            nc.tensor.matmul(
                                psv[(pt, kb)][:],
                                wv_s[:, kc, 128 * pt : 128 * (pt + 1)],
                                xt[:, 512 * kb : 512 * (kb + 1)],
                                start=(kc == 0), stop=(kc == 7),
                            )
                for pt in range(2):
                    for kb in range(NQB):
                        nc.vector.tensor_scalar_add(
                            vT_s[pt][:, 512 * kb : 512 * (kb + 1)],
                            psv[(pt, kb)][:], bv_s[:, pt : pt + 1],
                        )
                # PE transposes: 2 s-tiles (= 4 [128,128] blocks) per PSUM bank
                for sp in range(8):
                    pst = psP.tile([128, 512], F32, tag="pp", name=f"pst{sp}")
                    blk = 0
                    for k2 in range(2):
                        st = 2 * sp + k2
                        for pt in range(2):
                            nc.tensor.matmul(
                                pst[:, 256 * k2 + 128 * pt : 256 * k2 + 128 * pt + 128],
                                vT_s[pt][:, 128 * st : 128 * (st + 1)],
                                ident_s[:],
                                is_transpose=True,
                                start=(blk == 0), stop=(blk == 3),
                                skip_group_check=True,
                            )
                            blk += 1
                    for k2 in range(2):
                        st = 2 * sp + k2
                        dst = v_s[:, st, :].rearrange("p (h x) -> p h x", x=65)[:, :, 0:64]
                        nc.vector.tensor_copy(
                            dst,
                            pst[:, 256 * k2 : 256 * k2 + 256].rearrange("p (h x) -> p h x", x=64),
                        )

            # ---- output projection halves + split ReduceScatter ----
            rs_in = [dram.tile([S // 2, D], F32, name=f"rs_in{i}") for i in range(2)]
            rs_out = [dram.tile([256, D], F32, name=f"rs_out{i}") for i in range(2)]

            def emit_oproj_half(half, psO):
                # q rows [1024*half, 1024*half+1024) = ctx_s cols of qb-pair `half`
                for sl in range(8):
                    st = 8 * half + sl
                    po = psO.tile([128, 2, 512], F32, tag="po", bufs=4, name="po")
                    for nb in range(2):
                        for fc in range(2):
                            nc.tensor.matmul(
                                po[:, nb, :],
                                ctx_s[fc][:, 128 * st : 128 * (st + 1)],
                                wo_s[:, fc, 512 * nb : 512 * (nb + 1)],
                                start=(fc == 0), stop=(fc == 1),
                            )
                    ot = oout.tile([128, D], F32, tag="ot")
                    nc.vector.tensor_add(
                        ot[:].rearrange("p (n x) -> p n x", n=2), po[:], 
                        bo4_s[:].rearrange("p (n x) -> p n x", n=2))
                    nc.sync.dma_start(rs_in[half][128 * sl : 128 * (sl + 1), :], ot[:])
                if not os_mod.environ.get("BASS_SIM_NO_RS"):
                    import concourse.mybir as mybir_mod
                    nc.gpsimd.collective_compute(
                        "ReduceScatter", mybir_mod.AluOpType.add,
                        replica_groups=[[0, 1, 2, 3], [4, 5, 6, 7]],
                        ins=[rs_in[half].opt()], outs=[rs_out[half].opt()],
                    )
                    nc.sync.dma_start(
                        out[256 * half : 256 * (half + 1), :], rs_out[half][:])
                else:
                    nc.sync.dma_start(
                        out[256 * half : 256 * (half + 1), :],
                        rs_in[half][0:256, :])

            # ---- phase 2: attention, two (head, qb-pair) streams in flight ----
            first = True
            with tc.tile_pool(name="rbcp", bufs=2) as rbcp:
                for qbp in range(2):
                    psS = stack.enter_context(
                        tc.tile_pool(name=f"psS{qbp}", bufs=1, space="PSUM"))
                    psA = stack.enter_context(
                        tc.tile_pool(name=f"psA{qbp}", bufs=1, space="PSUM"))
                    nkt = 8 * qbp + 8  # k-tiles needed by this qb pair
                    for hp in range(2):
                        heads = (2 * hp, 2 * hp + 1)
                        ctx_ps = {
                            si: psA.tile([65, 1024], F32, tag="ctx", bufs=2, name=f"ctxps{si}")
                            for si in range(2)
                        }
                        for ki in range(nkt):
                            qbs = [qb for qb in (2 * qbp, 2 * qbp + 1) if qb >= ki // 4]
                            w = 512 * len(qbs)
                            for si, h in enumerate(heads):
                                pt, row = h // 2, 64 * (h % 2)
                                qT_h = qT_s[pt][row : row + 64, :]
                                kT_h = kT_s[pt][row : row + 64, :]
                                sc = psS.tile([128, 1024], F32, tag="sc", bufs=2, name=f"scps{si}")
                                for j, qb in enumerate(qbs):
                                    nc.tensor.matmul(
                                        sc[:, 512 * j : 512 * j + 512],
                                        kT_h[:, 128 * ki : 128 * (ki + 1)],
                                        qT_h[:, 512 * qb : 512 * (qb + 1)],
                                        start=True, stop=True,
                                    )
                                pr = probs.tile([128, 1024], F32R, tag="pr", name="pr")
                                nc.scalar.activation(
                                    out=pr[:, :w], in_=sc[:, :w], func=Exp, scale=0.125
                                )
                                prm = None
                                if qbs and qbs[0] == ki // 4:  # diagonal block present
                                    prm = probs.tile([128, 512], F32R, tag="prm", bufs=3, name="prm")
                                    nc.vector.tensor_mul(
                                        prm[:], pr[:, 0:512],
                                        masks_s[:, 512 * (ki % 4) : 512 * (ki % 4) + 512],
                                    )
                                for j, qb in enumerate(qbs):
                                    rhs = prm[:] if (j == 0 and prm is not None) else pr[:, 512 * j : 512 * j + 512]
                                    nc.tensor.matmul(
                                        ctx_ps[si][:, 512 * (qb - 2 * qbp) : 512 * (qb - 2 * qbp) + 512],
                                        v_s[:, ki, 65 * h : 65 * h + 65],
                                        rhs,
                                        start=(ki == 0), stop=(ki == 4 * qb + 3),
                                    )
                        if first:
                            # prefetch phase-3 constants during attention
                            nc.sync.dma_start(wo_s[:], wo.ap().rearrange("(c p) d -> p c d", p=128).bitcast(F32R))
                            nc.sync.dma_start(bo4_s[:], bo4.ap())
                            first = False
                        for si, h in enumerate(heads):
                            pt, row = h // 2, 64 * (h % 2)
                            # copy ctx psum out (frees banks), normalize off-PSUM
                            ctmp = rbcp.tile([65, 1024], F32, tag="ctmp", name="ctmp", bufs=2)
                            nc.vector.tensor_copy(ctmp[:], ctx_ps[si][:])
                            recip = small.tile([1, 1024], F32, tag="recip")
                            nc.vector.reciprocal(recip[:], ctmp[64:65, :])
                            rbc = rbcp.tile([64, 1024], F32, tag="rbc", bufs=2)
                            nc.gpsimd.partition_broadcast(rbc[:], recip[:])
                            nc.vector.tensor_mul(
                                ctx_s[pt][row : row + 64, 1024 * qbp : 1024 * (qbp + 1)],
                                ctmp[0:64, :], rbc[:],
                            )
                    if hp == 1:
                        stack.pop_all().close()  # close psS/psA for this qbp
                        with tc.tile_pool(name=f"psO{qbp}", bufs=4, space="PSUM") as psO:
                            emit_oproj_half(qbp, psO)


    nc.compile()
    return nc


def _prep_inputs(query, key_, value, w_q, b_q, w_k, b_k, w_v, b_v, w_o, b_o):
    """Build the 8 per-core input maps (host-side sharding / re-layout)."""
    f32 = np.float32
    # triangular mask patterns: t in 0..3, allowed iff j >= r + 128*t
    r = np.arange(128)[:, None]
    j = np.arange(512)[None, :]
    masks = np.concatenate(
        [(j >= r + 128 * t).astype(f32) for t in range(4)], axis=1
    )  # [128, 2048]
    ident = np.eye(128, dtype=f32)
    bo4 = np.broadcast_to(np.asarray(b_o, f32) / 4.0, (128, D)).copy()

    import os as os_mod
    if os_mod.environ.get("BASS_X_BF16"):
        import ml_dtypes
        xdt = ml_dtypes.bfloat16
    else:
        xdt = f32
    wqT = np.ascontiguousarray(np.asarray(w_q, f32).T)  # [D_in, D_out]
    wkT = np.ascontiguousarray(np.asarray(w_k, f32).T)
    wvT = np.ascontiguousarray(np.asarray(w_v, f32).T)
    woT = np.ascontiguousarray(np.asarray(w_o, f32).T)  # [D_in, D_out]

    xT = {}
    for g in range(B):
        xT[("q", g)] = np.ascontiguousarray(np.asarray(query[g], f32).T.astype(xdt))
        xT[("k", g)] = np.ascontiguousarray(np.asarray(key_[g], f32).T.astype(xdt))
        xT[("v", g)] = np.ascontiguousarray(np.asarray(value[g], f32).T.astype(xdt))

    in_maps = []
    for c in range(N_CORES):
        g, p = c // 4, c % 4
        fsel = slice(FPC * p, FPC * (p + 1))
        in_maps.append({
            "xq": xT[("q", g)],
            "xk": xT[("k", g)],
            "xv": xT[("v", g)],
            "wq": np.ascontiguousarray(wqT[:, fsel].astype(xdt)),
            "wk": np.ascontiguousarray(wkT[:, fsel].astype(xdt)),
            "wv": np.ascontiguousarray(wvT[:, fsel].astype(xdt)),
            "wo": np.ascontiguousarray(woT[fsel, :]),
            "bq": np.ascontiguousarray(np.asarray(b_q, f32)[fsel].reshape(FPC, 1)),
            "bk": np.ascontiguousarray(np.asarray(b_k, f32)[fsel].reshape(FPC, 1)),
            "bv": np.ascontiguousarray(np.asarray(b_v, f32)[fsel].reshape(FPC, 1)),
            "bo4": bo4,
            "masks": masks,
            "ident": ident,
        })
    return in_maps


def run(inputs, trace=False):
    from concourse.bass_utils import run_bass_kernel_spmd

    if "nc" not in _CACHE:
        _CACHE["nc"] = _build_nc()
    nc = _CACHE["nc"]
    in_maps = _prep_inputs(
        inputs["query"], inputs["key_"], inputs["value"],
        inputs["w_q"], inputs["b_q"], inputs["w_k"], inputs["b_k"],
        inputs["w_v"], inputs["b_v"], inputs["w_o"], inputs["b_o"],
    )
    res = run_bass_kernel_spmd(
        nc, in_maps, core_ids=list(range(N_CORES)), trace=trace,
    )
    out = np.empty((B, S, D), np.float32)
    for c in range(N_CORES):
        g, p = c // 4, c % 4
        # RS half i scatters q rows [1024*i + 256*p, 1024*i + 256*(p+1))
        out[g, 256 * p : 256 * (p + 1), :] = res.results[c]["out"][0:256]
        out[g, 1024 + 256 * p : 1024 + 256 * (p + 1), :] = res.results[c]["out"][256:512]
    return out, res


def kernel(**inputs):
    out, _ = run(inputs, trace=False)
    return out



# revision 15
# speedup vs baseline: 1.3520x; 1.3520x over previous
"""Causal multi-head attention on 8 Trainium2 NeuronCores.

Sharding: core c -> (batch g = c // 4, head-group p = c % 4, heads 4p..4p+3).
Each core projects Q/K/V for its batch with its 256 feature columns
(column-sharded w_q/w_k/w_v), runs causal attention for its 4 heads, computes
the partial output projection with its 256 rows of w_o, and a ReduceScatter
over each batch group sums the partials and hands every core its own 512-row
output shard.

v2 design (everything bf16 into the PE, fp32 accumulation):
- Exact 128-granularity causality: scores strips [128 kpos, q>=128*ki] per
  (head, k-tile), 53% of the S^2 matrix instead of 62.5%.
- Flipped AV: ctx[q-part, 65] += pr_block.T @ v_aug where v_aug is V in
  natural [kpos, feat] layout with an appended ones column -> softmax
  denominator lands in psum column 64, per-PARTITION normalize (cheap).
- V projected directly in natural layout (no PE transpose pass).
- b_k dropped entirely (softmax-shift invariant), b_v folded into the output
  bias host-side (bv @ w_o), b_q applied as the exp() per-partition bias via
  k.bq matvecs.
- Streamed tail: per q-tile normalize -> PE transpose -> output projection
  -> rs_in DMA, overlapped under the attention exp stream.
"""

import numpy as np

B, S, D, H = 2, 2048, 1024, 16
DK = D // H  # 64
N_CORES = 8
FPC = 256  # features per core
NKT = S // 128  # 16 k/q tiles

_CACHE = {}


def _build_nc():
    import os as os_mod
    import concourse.mybir as mybir
    import concourse.tile as tile
    from concourse import bacc

    F32 = mybir.dt.float32
    BF16 = mybir.dt.bfloat16
    Exp = mybir.ActivationFunctionType.Exp

    nc = bacc.Bacc("TRN2", target_bir_lowering=False, debug=False, num_devices=8)

    xq = nc.dram_tensor("xq", [D, S], BF16, kind="ExternalInput")
    xk = nc.dram_tensor("xk", [D, S], BF16, kind="ExternalInput")
    xv = nc.dram_tensor("xv", [D, S], BF16, kind="ExternalInput")
    wq = nc.dram_tensor("wq", [D, FPC], BF16, kind="ExternalInput")
    wk = nc.dram_tensor("wk", [D, FPC], BF16, kind="ExternalInput")
    wv = nc.dram_tensor("wv", [D, FPC], BF16, kind="ExternalInput")
    wo = nc.dram_tensor("wo", [FPC, D], BF16, kind="ExternalInput")
    bq8 = nc.dram_tensor("bq8", [FPC, 1], BF16, kind="ExternalInput")
    bo4 = nc.dram_tensor("bo4", [128, D], F32, kind="ExternalInput")
    tri = nc.dram_tensor("tri", [128, 128], BF16, kind="ExternalInput")
    ident = nc.dram_tensor("ident", [128, 128], BF16, kind="ExternalInput")
    out = nc.dram_tensor("out", [512, D], F32, kind="ExternalOutput")

    dbg = bool(os_mod.environ.get("BASS_DEBUG_DUMP"))
    if dbg:
        dbg_qT = nc.dram_tensor("dbg_qT", [128, 2, S], BF16, kind="ExternalOutput")
        dbg_kT = nc.dram_tensor("dbg_kT", [128, 2, S], BF16, kind="ExternalOutput")
        dbg_v = nc.dram_tensor("dbg_v", [128, NKT, 4 * 65], BF16, kind="ExternalOutput")
        dbg_kbq = nc.dram_tensor("dbg_kbq", [128, 4, NKT], F32, kind="ExternalOutput")
        dbg_ctxn = nc.dram_tensor("dbg_ctxn", [128, 2, NKT, 128], BF16, kind="ExternalOutput")
        dbg_ctxT = nc.dram_tensor("dbg_ctxT", [128, 2, S], BF16, kind="ExternalOutput")

    from contextlib import ExitStack
    with tile.TileContext(nc) as tc:
        with (
            tc.tile_pool(name="consts", bufs=1) as consts,
            tc.tile_pool(name="persist", bufs=1) as persist,
            tc.tile_pool(name="xin", bufs=3) as xin,
            tc.tile_pool(name="probs", bufs=18) as probs,
            tc.tile_pool(name="small", bufs=6) as small,
            tc.tile_pool(name="oout", bufs=2) as oout,
            tc.tile_pool(name="dram", bufs=1, space="DRAM") as dram,
        ):
            # ---- constant tiles ----
            wq_s = consts.tile([128, 8, FPC], BF16, tag="wq")
            wk_s = consts.tile([128, 8, FPC], BF16, tag="wk")
            wv_s = consts.tile([128, 8, FPC], BF16, tag="wv")
            wo_s = consts.tile([128, 2, D], BF16, tag="wo")
            bq8_s = consts.tile([128, 2], BF16, tag="bq8")
            bo4_s = consts.tile([128, D], F32, tag="bo4")
            tri_s = consts.tile([128, 128], BF16, tag="tri")
            ident_s = consts.tile([128, 128], BF16, tag="ident")


            # ---- persistent activations ----
            # qT/kT: [head-dim on partitions (2 heads: rows 0:64 / 64:128), S]
            qT_s = [persist.tile([128, S], BF16, tag=f"qT{i}", name=f"qT{i}") for i in range(2)]
            kT_s = [persist.tile([128, S], BF16, tag=f"kT{i}", name=f"kT{i}") for i in range(2)]
            # V natural layout + ones column: per ki, head h strip at 65h..65h+65
            v_s = persist.tile([128, NKT, 4 * 65], BF16, tag="v")
            nc.vector.memset(
                v_s[:].rearrange("p k (h x) -> p k h x", x=65)[:, :, :, 64:65], 1.0)
            kbq_s = persist.tile([128, 4, NKT], F32, tag="kbq")
            # normalized context, [q-part, 2 heads * 64 feat] per (qtile, pair)
            ctxn_s = [persist.tile([128, NKT, 128], BF16, tag=f"ctxn{i}", name=f"ctxn{i}")
                      for i in range(2)]
            # transposed normalized context [feat-part (2 heads), q] per pair
            ctxT_s = [persist.tile([128, S], BF16, tag=f"ctxT{i}", name=f"ctxT{i}")
                      for i in range(2)]
            # xv must stay fully resident (contraction over all 8 chunks per tile)
            xv_s = persist.tile([128, 8, S], BF16, tag="xv")

            # ---- DMA order on the serialized DMA resource ----
            # sync queue: wq, bq8, xq chunks, wk, tri/ident, xk chunks, wv, xv
            # scalar queue (emitted later, needed late): wo, bo4
            nc.sync.dma_start(wq_s[:], wq.ap().rearrange("(kc p) f -> p kc f", p=128))
            nc.sync.dma_start(bq8_s[:], bq8.ap().rearrange("(t p) o -> p (t o)", p=128))

            rs_in = [dram.tile([S // 2, D], F32, name=f"rs_in{i}") for i in range(2)]
            rs_out = [dram.tile([256, D], F32, name=f"rs_out{i}") for i in range(2)]

            # ================= phase A: Q/K projections =================
            with tc.tile_pool(name="psProj", bufs=8, space="PSUM") as psP:
                def proj_pass(x_dram, w_sb, dst, name):
                    ps = {(pt, qb): psP.tile([128, 512], F32, tag="pp",
                                             name=f"ps{name}{pt}{qb}")
                          for pt in range(2) for qb in range(4)}
                    for kc in range(8):
                        xt = xin.tile([128, S], BF16, tag="x", name=f"x{name}{kc}")
                        nc.sync.dma_start(xt[:], x_dram[128 * kc: 128 * (kc + 1), :])
                        for pt in range(2):
                            for qb in range(4):
                                nc.tensor.matmul(
                                    ps[(pt, qb)][:],
                                    w_sb[:, kc, 128 * pt: 128 * (pt + 1)],
                                    xt[:, 512 * qb: 512 * (qb + 1)],
                                    start=(kc == 0), stop=(kc == 7),
                                )
                    for pt in range(2):
                        for qb in range(4):
                            nc.vector.tensor_copy(
                                dst[pt][:, 512 * qb: 512 * (qb + 1)], ps[(pt, qb)][:])

                proj_pass(xq, wq_s, qT_s, "q")
                nc.sync.dma_start(wk_s[:], wk.ap().rearrange("(kc p) f -> p kc f", p=128))
                nc.sync.dma_start(tri_s[:], tri.ap())
                nc.sync.dma_start(ident_s[:], ident.ap())
                proj_pass(xk, wk_s, kT_s, "k")
                nc.sync.dma_start(wv_s[:], wv.ap().rearrange("(kc p) f -> p kc f", p=128))
                for kc in range(8):
                    nc.sync.dma_start(xv_s[:, kc, :], xv[128 * kc: 128 * (kc + 1), :])

                # kbq[h, ki] column: (kT_h . bq_h/8) per kpos partition (exp bias)
                kbq_ps = psP.tile([128, 512], F32, tag="pp", name="kbq_ps")
                for h in range(4):
                    pt, row = h // 2, 64 * (h % 2)
                    for ki in range(NKT):
                        nc.tensor.matmul(
                            kbq_ps[:, 16 * h + ki: 16 * h + ki + 1],
                            kT_s[pt][row: row + 64, 128 * ki: 128 * (ki + 1)],
                            bq8_s[row: row + 64, pt: pt + 1],
                            start=True, stop=True,
                        )
                nc.vector.tensor_copy(
                    kbq_s[:].rearrange("p h k -> p (h k)"), kbq_ps[:, 0:64])

            # wo/bo4 arrive during attention (sync queue FIFO: after x traffic)
            nc.sync.dma_start(wo_s[:], wo.ap().rearrange("(c p) d -> p c d", p=128))
            nc.sync.dma_start(bo4_s[:], bo4.ap())

            # ================= phase B: attention + streamed oproj =================
            v_emitted = [0]

            def emit_v_bundle(psA):
                """Project V for one k-tile into natural layout (1 psum bank).
                An accumulation group must own its whole psum bank: start=True
                marks the full 2KB bank pending-zero."""
                ki = v_emitted[0]
                if ki >= NKT:
                    return
                v_emitted[0] += 1
                vp = psA.tile([128, 256], F32, tag="sa", name=f"vps{ki}")
                for kc in range(8):
                    nc.tensor.matmul(
                        vp[:],
                        xv_s[:, kc, 128 * ki: 128 * (ki + 1)],
                        wv_s[:, kc, :],
                        start=(kc == 0), stop=(kc == 7),
                    )
                nc.vector.tensor_copy(
                    v_s[:, ki, :].rearrange("p (h x) -> p h x", x=65)[:, :, 0:64],
                    vp[:].rearrange("p (h x) -> p h x", x=64),
                )

            with (
                tc.tile_pool(name="psS", bufs=2, space="PSUM") as psS,
                tc.tile_pool(name="psC", bufs=2, space="PSUM") as psC,
                tc.tile_pool(name="psA", bufs=1, space="PSUM") as psA,
                tc.tile_pool(name="psB", bufs=1, space="PSUM") as psB,
            ):
                def emit_oproj(qi, eng_idx):
                    """po = ctxT0.T @ wo[0] + ctxT1.T @ wo[1] (+bo4) for q-tile qi."""
                    ot = oout.tile([128, D], F32, tag="ot")
                    for dh in range(2):
                        po = psA.tile([128, 512], F32, tag="sa", name=f"po{qi}{dh}")
                        for p in range(2):
                            nc.tensor.matmul(
                                po[:],
                                ctxT_s[p][:, 128 * qi: 128 * (qi + 1)],
                                wo_s[:, p, 512 * dh: 512 * (dh + 1)],
                                start=(p == 0), stop=(p == 1),
                            )
                        nc.vector.tensor_add(
                            ot[:, 512 * dh: 512 * (dh + 1)], po[:],
                            bo4_s[:, 512 * dh: 512 * (dh + 1)])
                    half, sl = qi // 8, qi % 8
                    nc.gpsimd.dma_start(
                        rs_in[half][128 * sl: 128 * (sl + 1), :], ot[:])
                    if sl == 7:
                        if not os_mod.environ.get("BASS_SIM_NO_RS"):
                            nc.gpsimd.collective_compute(
                                "ReduceScatter", mybir.AluOpType.add,
                                replica_groups=[[0, 1, 2, 3], [4, 5, 6, 7]],
                                ins=[rs_in[half].opt()], outs=[rs_out[half].opt()],
                            )
                            nc.sync.dma_start(
                                out[256 * half: 256 * (half + 1), :], rs_out[half][:])
                        else:
                            nc.sync.dma_start(
                                out[256 * half: 256 * (half + 1), :],
                                rs_in[half][0:256, :])

                for pair in range(2):
                    heads = (2 * pair, 2 * pair + 1)
                    for qg in range(2):
                        q0, q1 = 1024 * qg, 1024 * (qg + 1)
                        for h in heads:
                            pt, row = h // 2, 64 * (h % 2)
                            prs = {}  # resident pr strips for this head
                            for ki in range(8 * qg + 8):
                                qstart = max(128 * ki, q0)
                                w = q1 - qstart
                                sc = psS.tile([128, 1024], F32, tag="sc", name="sc")
                                for c0 in range(0, w, 512):
                                    cw = min(512, w - c0)
                                    nc.tensor.matmul(
                                        sc[:, c0: c0 + cw],
                                        kT_s[pt][row: row + 64, 128 * ki: 128 * (ki + 1)],
                                        qT_s[pt][row: row + 64, qstart + c0: qstart + c0 + cw],
                                        start=True, stop=True,
                                    )
                                pr = probs.tile([128, 1024], BF16, tag="pr", name="pr")
                                prs[ki] = (pr, qstart)
                                nc.scalar.activation(
                                    out=pr[:, :w], in_=sc[:, :w], func=Exp,
                                    bias=kbq_s[:, h, ki: ki + 1], scale=0.125,
                                )
                                if 128 * ki >= q0:  # diagonal block: causal mask
                                    nc.vector.tensor_mul(
                                        pr[:, 0:128], pr[:, 0:128], tri_s[:])
                                # V bundles must be emitted before the AVs that
                                # consume them (engine order = emission order)
                                if pair == 0 and h == heads[0]:
                                    while v_emitted[0] <= ki:
                                        emit_v_bundle(psA)
                                if ki < 8 * qg:
                                    continue
                                # q-tile qi == ki has all strips available:
                                # accumulate its ctx in a dedicated psum bank
                                qi = ki
                                ctx = psC.tile([128, 65], F32, tag="ctx",
                                               name=f"ctx{pair}{qg}{h}{qi}")
                                for k2 in range(qi + 1):
                                    pr2, qs2 = prs[k2]
                                    off = 128 * qi - qs2
                                    nc.tensor.matmul(
                                        ctx[:],
                                        pr2[:, off: off + 128],
                                        v_s[:, k2, 65 * h: 65 * h + 65],
                                        start=(k2 == 0), stop=(k2 == qi),
                                    )
                                rc = small.tile([128, 1], F32, tag="rc")
                                nc.vector.reciprocal(rc[:], ctx[:, 64:65])
                                nc.vector.tensor_scalar_mul(
                                    ctxn_s[pair][:, qi, row: row + 64],
                                    ctx[:, 0:64], rc[:])
                                if h == heads[1]:
                                    tp = psB.tile([128, 128], BF16, tag="sb",
                                                  name=f"tp{pair}{qi}")
                                    nc.tensor.matmul(
                                        tp[:], ctxn_s[pair][:, qi, :], ident_s[:],
                                        is_transpose=True, start=True, stop=True,
                                        skip_group_check=True,
                                    )
                                    nc.vector.tensor_copy(
                                        ctxT_s[pair][:, 128 * qi: 128 * (qi + 1)], tp[:])
                                    if pair == 1:
                                        emit_oproj(qi, qi)


            if dbg:
                for i in range(2):
                    nc.sync.dma_start(dbg_qT[:, i, :], qT_s[i][:])
                    nc.sync.dma_start(dbg_kT[:, i, :], kT_s[i][:])
                    nc.sync.dma_start(dbg_ctxn[:, i, :, :], ctxn_s[i][:])
                    nc.sync.dma_start(dbg_ctxT[:, i, :], ctxT_s[i][:])
                nc.sync.dma_start(dbg_v[:], v_s[:])
                nc.sync.dma_start(dbg_kbq[:], kbq_s[:])

    nc.compile()
    return nc


def _prep_inputs(query, key_, value, w_q, b_q, w_k, b_k, w_v, b_v, w_o, b_o):
    """Build the 8 per-core input maps (host-side sharding / re-layout)."""
    import ml_dtypes
    f32 = np.float32
    bf16 = ml_dtypes.bfloat16

    r = np.arange(128)[:, None]
    j = np.arange(128)[None, :]
    tri = (j >= r).astype(bf16)  # allowed (q >= k) within diagonal block
    ident = np.eye(128, dtype=bf16)

    wqT = np.ascontiguousarray(np.asarray(w_q, f32).T)  # [D_in, D_out]
    wkT = np.ascontiguousarray(np.asarray(w_k, f32).T)
    wvT = np.ascontiguousarray(np.asarray(w_v, f32).T)
    woT = np.ascontiguousarray(np.asarray(w_o, f32).T)  # [D_in(=feat), D_out]
    b_q = np.asarray(b_q, f32)
    b_v = np.asarray(b_v, f32)
    b_o = np.asarray(b_o, f32)

    xT = {}
    for g in range(B):
        xT[("q", g)] = np.ascontiguousarray(np.asarray(query[g], f32).T.astype(bf16))
        xT[("k", g)] = np.ascontiguousarray(np.asarray(key_[g], f32).T.astype(bf16))
        xT[("v", g)] = np.ascontiguousarray(np.asarray(value[g], f32).T.astype(bf16))

    in_maps = []
    for c in range(N_CORES):
        g, p = c // 4, c % 4
        fsel = slice(FPC * p, FPC * (p + 1))
        # b_v folded into the output bias: ctx_norm contains +b_v, so
        # out partial += b_v[fsel] @ w_o.T[fsel, :]; b_o/4 spread over 4 cores.
        bo_eff = b_o / 4.0 + b_v[fsel] @ woT[fsel, :]
        bo4 = np.broadcast_to(bo_eff.astype(f32), (128, D)).copy()
        in_maps.append({
            "xq": xT[("q", g)],
            "xk": xT[("k", g)],
            "xv": xT[("v", g)],
            "wq": np.ascontiguousarray(wqT[:, fsel].astype(bf16)),
            "wk": np.ascontiguousarray(wkT[:, fsel].astype(bf16)),
            "wv": np.ascontiguousarray(wvT[:, fsel].astype(bf16)),
            "wo": np.ascontiguousarray(woT[fsel, :].astype(bf16)),
            "bq8": np.ascontiguousarray(
                (b_q[fsel] / 8.0).reshape(FPC, 1).astype(bf16)),
            "bo4": bo4,
            "tri": tri,
            "ident": ident,
        })
    return in_maps


def run(inputs, trace=False):
    from concourse.bass_utils import run_bass_kernel_spmd

    if "nc" not in _CACHE:
        _CACHE["nc"] = _build_nc()
    nc = _CACHE["nc"]
    in_maps = _prep_inputs(
        inputs["query"], inputs["key_"], inputs["value"],
        inputs["w_q"], inputs["b_q"], inputs["w_k"], inputs["b_k"],
        inputs["w_v"], inputs["b_v"], inputs["w_o"], inputs["b_o"],
    )
    res = run_bass_kernel_spmd(
        nc, in_maps, core_ids=list(range(N_CORES)), trace=trace,
    )
    out = np.empty((B, S, D), np.float32)
    for c in range(N_CORES):
        g, p = c // 4, c % 4
        # RS half i scatters q rows [1024*i + 256*p, 1024*i + 256*(p+1))
        out[g, 256 * p: 256 * (p + 1), :] = res.results[c]["out"][0:256]
        out[g, 1024 + 256 * p: 1024 + 256 * (p + 1), :] = res.results[c]["out"][256:512]
    return out, res


def kernel(**inputs):
    out, _ = run(inputs, trace=False)
    return out
